# revision 9
# baseline (speedup 1.0000x reference)
"""Distance-aware masking kernel v3 for Trainium2 (8 NeuronCores).

mask[i,j,:] = W2 @ relu(W1 @ [r_i - c_j, |r_i - c_j|] + b1) + b2,
N=4096 nodes, rows sharded across 8 cores (512 rows each).

v3 pipeline (per core), designed against the TRN2 cost model:
  1. S-matmul pairs (PE, bf16 split2, K=15): dist^2(i,j)+eps for 64 rows
     per tile (2 groups of 32 at partition offsets 0/64, basis slots at
     32-35/96-99), two tiles per [100,1024] 2-bank psum.
  2. ACT sqrt [100,1024]: psum -> rt_big bf16 (one 8KB/partition tile per
     j-tile holding all 8 S-tiles). Basis rows [1,g0,g1,g2] land at
     partitions 32-35/96-99 via TWO DMAs per j-tile (after the 4th sqrt).
  3. u-matmul (PE, K=36): u[4*di+m, j] = a_m*dist + alpha_m(i) - g_m(j).
     Group slots read rhs [0:36]/[64:100] (the only legal 36-row windows:
     weight APs cannot cross a 64-partition boundary from offset 32),
     u_lhsT rows at matching partitions.
  4. relu: ps_u -> h f16 [128,512] (DVE mostly; GPSIMD cannot read PSUM).
  5. mix matmuls (PE, f16): block-diagonal W2 -> out[8*di+o, j] in a
     [128,1024] 2-bank psum (2 bufs).
  6. copies psum -> t_o f16 split ACT[0:672]/DVE[672:1024]; DMA to DRAM
     scratch (row = 8*i + o) per 4 groups on the Pool queue.
Back-stages run with a 2-group lag behind the u-matmuls so every PE
instruction's producers complete >=1.3us ahead - the PE stays
continuously busy and holds the 2.4 GHz p-state. Host casts to f32,
adds b2, patches the exact diagonal.
"""

import sys

sys.path.insert(0, "/opt/trn_rl_repo")

import numpy as np
import ml_dtypes

N = 4096
N_CORES = 8
ROWS = N // N_CORES          # 512 rows per core
GR = 32                      # rows per group
NG = ROWS // GR              # 16 groups per j-tile
J = 512                      # j-tile width
NJ = N // J                  # 8 j-tiles
NT = 8                       # S-tiles per j-tile, 2 groups each
EPS = 3e-4                   # dist^2 floor; protects sqrt from cancellation
DIM = 3
DIM_OUT = 8

_BF = ml_dtypes.bfloat16

_CACHE = {}

# partition offsets of the 2 group slots / their basis slots in an S-tile
_SLOT_OFF = (0, 64)
_BAS = 32


def _split2(x):
    hi = x.astype(_BF)
    lo = (x - hi.astype(np.float32)).astype(_BF)
    return hi, lo


def _build_program():
    import concourse.bass as bass  # noqa: F401
    import concourse.mybir as mybir
    import concourse.tile as tile
    from concourse import bacc

    nc = bacc.Bacc("TRN2", target_bir_lowering=False, num_devices=N_CORES)

    f16 = mybir.dt.float16
    bf16 = mybir.dt.bfloat16
    f32 = mybir.dt.float32

    s_lhsT = nc.dram_tensor("s_lhsT", [15, NT * 128], bf16, kind="ExternalInput").ap()
    s_rhs = nc.dram_tensor("s_rhs", [15, N], bf16, kind="ExternalInput").ap()
    # basis rows [1,g0,g1,g2] per j-tile, replicated over the 8 S-tile
    # column blocks; rows 0-3 for partition slot 32-35, rows 4-7 for 96-99
    basisD = nc.dram_tensor("basisD", [8, NJ * NT * J], bf16, kind="ExternalInput").ap()
    u_lhsT = nc.dram_tensor("u_lhsT", [100, NG * 128], bf16, kind="ExternalInput").ap()
    mixw = nc.dram_tensor("mixw", [128, 128], f16, kind="ExternalInput").ap()
    scratch = nc.dram_tensor("scratch", [N, N], f16, kind="ExternalOutput").ap()

    NSTEP = NJ * NG  # 128 global group steps

    with tile.TileContext(nc) as tc:
        with tc.tile_pool(name="const", bufs=1) as cp, \
             tc.tile_pool(name="rhsp", bufs=2) as rp, \
             tc.tile_pool(name="hp", bufs=4) as hp, \
             tc.tile_pool(name="outp", bufs=3) as op, \
             tc.tile_pool(name="pss", bufs=1, space="PSUM") as pss, \
             tc.tile_pool(name="psu", bufs=2, space="PSUM") as psu, \
             tc.tile_pool(name="psm", bufs=2, space="PSUM") as psm:

            t_s_lhsT = cp.tile([15, NT * 128], bf16, tag="t_s_lhsT")
            nc.sync.dma_start(t_s_lhsT[:], s_lhsT)
            t_s_rhs = cp.tile([15, N], bf16, tag="t_s_rhs")
            nc.sync.dma_start(t_s_rhs[:], s_rhs)
            t_u_lhsT = cp.tile([100, NG * 128], bf16, tag="t_u_lhsT")
            nc.sync.dma_start(t_u_lhsT[:], u_lhsT)
            t_mixw = cp.tile([128, 128], f16, tag="t_mixw")
            nc.sync.dma_start(t_mixw[:], mixw)

            rt_cur = {}

            def rt_alloc(jt):
                rt_cur[jt] = rp.tile([100, NT * J], bf16, tag="rt", name="rt")

            def s_pair(jt, p):
                """S-matmuls for tiles 2p,2p+1 of jt + one [100,1024] sqrt."""
                jcol = slice(jt * J, (jt + 1) * J)
                ps_s = pss.tile([100, 2 * J], f32, tag="ps_s")
                for h in range(2):
                    t = 2 * p + h
                    nc.tensor.matmul(
                        ps_s[:, h * J:(h + 1) * J],
                        t_s_lhsT[:, t * 128:t * 128 + 100],
                        t_s_rhs[:, jcol],
                        start=True, stop=True,
                    )
                rt = rt_cur[jt]
                nc.scalar.activation(
                    rt[:, 2 * p * J:(2 * p + 2) * J], ps_s[:],
                    mybir.ActivationFunctionType.Sqrt,
                )

            def basis_dmas(jt):
                rt = rt_cur[jt]
                bcol = slice(jt * NT * J, (jt + 1) * NT * J)
                nc.sync.dma_start(rt[32:36, :], basisD[0:4, bcol])
                nc.sync.dma_start(rt[96:100, :], basisD[4:8, bcol])

            h_tiles = {}
            to_cur = [None]

            def front(n):
                """u-matmul + relu for global group n."""
                jt, g = divmod(n, NG)
                t, s = divmod(g, 2)
                rt = rt_cur[jt]
                off = _SLOT_OFF[s]
                ps_u = psu.tile([128, J], f32, tag="ps_u")
                nc.tensor.matmul(
                    ps_u[:],
                    t_u_lhsT[off:off + 36, g * 128:(g + 1) * 128],
                    rt[off:off + 36, t * J:(t + 1) * J],
                    start=True, stop=True,
                )
                t_h = hp.tile([128, J], f16, tag="t_h")
                if g == 14:
                    # 1-in-16 relus on ACT to balance the engines
                    nc.scalar.activation(
                        t_h[:], ps_u[:], mybir.ActivationFunctionType.Relu
                    )
                else:
                    nc.vector.tensor_scalar_max(t_h[:], ps_u[:], 0.0)
                h_tiles[n] = t_h

            def back(n):
                """mix matmuls + 672/352 copies (+ DMA per 4 groups)."""
                jt, g = divmod(n, NG)
                t_h = h_tiles.pop(n)
                if g % 4 == 0:
                    to_cur[0] = op.tile([128, 8 * J], f16, tag="t_o", name="t_o")
                t_o = to_cur[0]
                ps_o = psm.tile([128, 2 * J], f32, tag="ps_o")
                for w in range(2):
                    pr = slice(64 * w, 64 * w + 64)
                    nc.tensor.matmul(
                        ps_o[:, w * J:(w + 1) * J], t_mixw[pr, :], t_h[pr, :],
                        start=True, stop=True,
                    )
                c0 = (g % 4) * 2 * J
                nc.scalar.copy(t_o[:, c0:c0 + 672], ps_o[:, 0:672])
                nc.vector.tensor_copy(t_o[:, c0 + 672:c0 + 1024], ps_o[:, 672:1024])
                if g % 4 == 3:
                    jcol = slice(jt * J, (jt + 1) * J)
                    row0 = (g - 3) * 8 * GR
                    dview = scratch[row0:row0 + 1024, jcol].rearrange(
                        "(g w p) j -> p g w j", g=4, w=2
                    )
                    src = t_o[:].rearrange("p (g w j) -> p g w j", g=4, w=2)
                    nc.gpsimd.dma_start(dview, src)

            # prologue: j-tile 0's S stages + basis rows
            rt_alloc(0)
            for p in range(4):
                s_pair(0, p)
            basis_dmas(0)

            # steady state: fronts lead backs by 2 groups; jt+1's S pairs
            # and basis DMAs interleave at fixed steps
            for n in range(NSTEP + 2):
                if n < NSTEP:
                    jt, g = divmod(n, NG)
                    if jt + 1 < NJ:
                        if g == 1:
                            rt_alloc(jt + 1)
                        if g in (1, 5, 9, 12):
                            s_pair(jt + 1, {1: 0, 5: 1, 9: 2, 12: 3}[g])
                        if g == 13:
                            basis_dmas(jt + 1)
                    front(n)
                if n >= 2:
                    back(n - 2)

    nc.compile()
    return nc


def _host_inputs(node_coords, W1, b1, W2, b2):
    coords = node_coords.astype(np.float32)
    W1 = W1.astype(np.float32)
    b1 = b1.astype(np.float32)
    W2 = W2.astype(np.float32)

    a = W1[:, 3]                       # [3] dist coefficients
    Wc = W1[:, :3]                     # [3,3] coord coefficients
    g = coords @ Wc.T                  # [N,3]
    c2 = (coords * coords).sum(1)      # [N]

    # ---- S rhs: j-side basis rows [cx, cy, cz, |c|^2, 1], split2 ----
    R = np.zeros((5, N), np.float32)
    R[0:3] = coords.T
    R[3] = c2
    R[4] = 1.0
    Rh, Rl = _split2(R)
    s_rhs = np.vstack([Rh, Rl, Rh])                   # [15, N]

    # ---- basis rows for the u-matmul rhs: [1, g0, g1, g2], replicated
    # over the 8 S-tile column blocks; rows 4-7 duplicate for slot 96 ----
    basis = np.zeros((4, N), np.float32)
    basis[0] = 1.0
    basis[1:4] = g.T
    basisD = np.empty((8, NJ * NT * J), np.float32)
    bv = basisD.reshape(2, 4, NJ, NT, J)
    for t in range(NT):
        bv[:, :, :, t, :] = basis.reshape(4, NJ, J)
    basisD = basisD.astype(_BF)

    # ---- mix weights (block-diagonal W2), two 64-row windows ----
    mixw = np.zeros((128, 128), np.float32)
    for w in range(2):
        for di in range(16):
            for m in range(3):
                mixw[64 * w + 4 * di + m, 8 * di + 0:8 * di + 8] = W2[:, m]
    mixw = mixw.astype(np.float16)

    in_maps = []
    for c in range(N_CORES):
        r = coords[c * ROWS:(c + 1) * ROWS]          # [512,3]
        r2 = (r * r).sum(1)                          # [512]
        alpha = r @ Wc.T + b1                        # [512,3]

        # ---- S lhsT: tile t holds groups 2t,2t+1 at col offsets
        # 0/36 within its 128-col block; basis cols 32-35 stay zero ----
        L = np.zeros((5, NT * 128), np.float32)
        for t in range(NT):
            for s in range(2):
                gi = 2 * t + s
                i0 = gi * GR
                cc = t * 128 + _SLOT_OFF[s]
                rr = r[i0:i0 + GR]                   # [32,3]
                L[0:3, cc:cc + GR] = -2.0 * rr.T
                L[3, cc:cc + GR] = 1.0
                L[4, cc:cc + GR] = r2[i0:i0 + GR] + EPS
        Lh, Ll = _split2(L)
        s_lhsT = np.vstack([Lh, Lh, Ll])             # [15, NT*128]

        # ---- u lhsT [100, NG*128]: rows live at the group's slot
        # partitions so lhsT/rhs share a base partition.  Per slot:
        # dist rows off..off+31 (a), ones row off+32 (alpha), g rows
        # off+33..35 (-1) ----
        u = np.zeros((100, NG * 128), np.float32)
        for gi in range(NG):
            off = _SLOT_OFF[gi % 2]
            i0 = gi * GR
            for di in range(GR):
                for m in range(3):
                    p = gi * 128 + 4 * di + m
                    u[off + di, p] = a[m]
                    u[off + 32, p] = alpha[i0 + di, m]
                    u[off + 33 + m, p] = -1.0
        u_lhsT = u.astype(_BF)

        in_maps.append({
            "s_lhsT": np.ascontiguousarray(s_lhsT),
            "s_rhs": np.ascontiguousarray(s_rhs),
            "basisD": np.ascontiguousarray(basisD),
            "u_lhsT": np.ascontiguousarray(u_lhsT),
            "mixw": mixw,
        })
    return in_maps


def kernel(node_coords, W1, b1, W2, b2):
    from concourse.bass_utils import run_bass_kernel_spmd

    if "nc" not in _CACHE:
        _CACHE["nc"] = _build_program()
    nc = _CACHE["nc"]

    in_maps = _host_inputs(node_coords, W1, b1, W2, b2)
    res = run_bass_kernel_spmd(nc, in_maps, core_ids=list(range(N_CORES)))
    _CACHE["last_res"] = res

    out = np.empty((N, N, DIM_OUT), np.float32)
    for c in range(N_CORES):
        sc = res.results[c]["scratch"]                   # [4096, 4096] f16
        blk = sc.astype(np.float32).reshape(ROWS, DIM_OUT, N).transpose(0, 2, 1)
        out[c * ROWS:(c + 1) * ROWS] = blk

    b2f = b2.astype(np.float32)
    if np.any(b2f):
        out += b2f

    # exact diagonal (device path has an eps floor under the sqrt)
    h_diag = np.maximum(b1.astype(np.float32), 0.0)
    diag = W2.astype(np.float32) @ h_diag + b2f
    idx = np.arange(N)
    out[idx, idx, :] = diag

    return out


# revision 11
# speedup vs baseline: 1.0555x; 1.0555x over previous
"""Distance-aware masking kernel v3 for Trainium2 (8 NeuronCores).

mask[i,j,:] = W2 @ relu(W1 @ [r_i - c_j, |r_i - c_j|] + b1) + b2,
N=4096 nodes, rows sharded across 8 cores (512 rows each).

v3 pipeline (per core), designed against the TRN2 cost model:
  1. S-matmul pairs (PE, bf16 split2, K=15): dist^2(i,j)+eps for 64 rows
     per tile (2 groups of 32 at partition offsets 0/64, basis slots at
     32-35/96-99), two tiles per [100,1024] 2-bank psum.
  2. ACT sqrt [100,1024]: psum -> rt_big bf16 (one 8KB/partition tile per
     j-tile holding all 8 S-tiles). Basis rows [1,g0,g1,g2] land at
     partitions 32-35/96-99 via TWO DMAs per j-tile (after the 4th sqrt).
  3. u-matmul (PE, K=36): u[4*di+m, j] = a_m*dist + alpha_m(i) - g_m(j).
     Group slots read rhs [0:36]/[64:100] (the only legal 36-row windows:
     weight APs cannot cross a 64-partition boundary from offset 32),
     u_lhsT rows at matching partitions.
  4. relu: ps_u -> h f16 [128,512] (DVE mostly; GPSIMD cannot read PSUM).
  5. mix matmuls (PE, f16): block-diagonal W2 -> out[8*di+o, j] in a
     [128,1024] 2-bank psum (2 bufs).
  6. copies psum -> t_o f16 split ACT[0:672]/DVE[672:1024]; DMA to DRAM
     scratch (row = 8*i + o) per 4 groups on the Pool queue.
Back-stages run with a 2-group lag behind the u-matmuls so every PE
instruction's producers complete >=1.3us ahead - the PE stays
continuously busy and holds the 2.4 GHz p-state. Host casts to f32,
adds b2, patches the exact diagonal.
"""

import sys

sys.path.insert(0, "/opt/trn_rl_repo")

import numpy as np
import ml_dtypes

N = 4096
N_CORES = 8
ROWS = N // N_CORES          # 512 rows per core
GR = 32                      # rows per group
NG = ROWS // GR              # 16 groups per j-tile
J = 512                      # j-tile width
NJ = N // J                  # 8 j-tiles
NT = 8                       # S-tiles per j-tile, 2 groups each
EPS = 3e-4                   # dist^2 floor; protects sqrt from cancellation
DIM = 3
DIM_OUT = 8

_BF = ml_dtypes.bfloat16

_CACHE = {}

# partition offsets of the 2 group slots / their basis slots in an S-tile
_SLOT_OFF = (0, 64)
_BAS = 32


def _split2(x):
    hi = x.astype(_BF)
    lo = (x - hi.astype(np.float32)).astype(_BF)
    return hi, lo


def _build_program():
    import concourse.bass as bass  # noqa: F401
    import concourse.mybir as mybir
    import concourse.tile as tile
    from concourse import bacc

    nc = bacc.Bacc("TRN2", target_bir_lowering=False, num_devices=N_CORES)

    f16 = mybir.dt.float16
    bf16 = mybir.dt.bfloat16
    f32 = mybir.dt.float32

    s_lhsT = nc.dram_tensor("s_lhsT", [15, NT * 128], bf16, kind="ExternalInput").ap()
    s_rhs = nc.dram_tensor("s_rhs", [15, N], bf16, kind="ExternalInput").ap()
    # basis rows [1,g0,g1,g2] per j-tile, replicated over the 8 S-tile
    # column blocks; rows 0-3 for partition slot 32-35, rows 4-7 for 96-99
    basisD = nc.dram_tensor("basisD", [8, NJ * NT * J], bf16, kind="ExternalInput").ap()
    u_lhsT = nc.dram_tensor("u_lhsT", [100, NG * 128], bf16, kind="ExternalInput").ap()
    mixw = nc.dram_tensor("mixw", [128, 128], f16, kind="ExternalInput").ap()
    scratch = nc.dram_tensor("scratch", [N, N], f16, kind="ExternalOutput").ap()

    NSTEP = NJ * NG  # 128 global group steps

    with tile.TileContext(nc) as tc:
        with tc.tile_pool(name="const", bufs=1) as cp, \
             tc.tile_pool(name="rhsp", bufs=2) as rp, \
             tc.tile_pool(name="hp", bufs=4) as hp, \
             tc.tile_pool(name="outp", bufs=3) as op, \
             tc.tile_pool(name="pss", bufs=1, space="PSUM") as pss, \
             tc.tile_pool(name="psu", bufs=2, space="PSUM") as psu, \
             tc.tile_pool(name="psm", bufs=2, space="PSUM") as psm:

            t_s_lhsT = cp.tile([15, NT * 128], bf16, tag="t_s_lhsT")
            nc.sync.dma_start(t_s_lhsT[:], s_lhsT)
            t_s_rhs = cp.tile([15, N], bf16, tag="t_s_rhs")
            nc.sync.dma_start(t_s_rhs[:], s_rhs)
            t_u_lhsT = cp.tile([100, NG * 128], bf16, tag="t_u_lhsT")
            nc.sync.dma_start(t_u_lhsT[:], u_lhsT)
            t_mixw = cp.tile([128, 128], f16, tag="t_mixw")
            nc.sync.dma_start(t_mixw[:], mixw)

            rt_cur = {}

            def rt_alloc(jt):
                rt_cur[jt] = rp.tile([100, NT * J], bf16, tag="rt", name="rt")

            def s_pair_mm(jt, p):
                """S-matmuls for tiles 2p,2p+1 of jt into a [100,1024] psum."""
                jcol = slice(jt * J, (jt + 1) * J)
                ps_s = pss.tile([100, 2 * J], f32, tag="ps_s")
                for h in range(2):
                    t = 2 * p + h
                    nc.tensor.matmul(
                        ps_s[:, h * J:(h + 1) * J],
                        t_s_lhsT[:, t * 128:t * 128 + 100],
                        t_s_rhs[:, jcol],
                        start=True, stop=True,
                    )
                return ps_s

            def s_pair_sqrt(jt, p, ps_s):
                rt = rt_cur[jt]
                nc.scalar.activation(
                    rt[:, 2 * p * J:(2 * p + 2) * J], ps_s[:],
                    mybir.ActivationFunctionType.Sqrt,
                )

            def basis_dmas(jt, p=None):
                """Basis-row DMAs; p=None covers the whole j-tile, else one
                S-tile pair (used to pipeline the jt-0 prologue)."""
                rt = rt_cur[jt]
                if p is None:
                    cl, ch = 0, NT * J
                else:
                    cl, ch = 2 * p * J, (2 * p + 2) * J
                bcol = slice(jt * NT * J + cl, jt * NT * J + ch)
                nc.sync.dma_start(rt[32:36, cl:ch], basisD[0:4, bcol])
                nc.sync.dma_start(rt[96:100, cl:ch], basisD[4:8, bcol])

            h_tiles = {}
            to_cur = [None]

            def front(n):
                """u-matmul + relu for global group n."""
                jt, g = divmod(n, NG)
                t, s = divmod(g, 2)
                rt = rt_cur[jt]
                off = _SLOT_OFF[s]
                ps_u = psu.tile([128, J], f32, tag="ps_u")
                nc.tensor.matmul(
                    ps_u[:],
                    t_u_lhsT[off:off + 36, g * 128:(g + 1) * 128],
                    rt[off:off + 36, t * J:(t + 1) * J],
                    start=True, stop=True,
                )
                t_h = hp.tile([128, J], f16, tag="t_h")
                if g == 14:
                    # 1-in-16 relus on ACT to balance the engines
                    nc.scalar.activation(
                        t_h[:], ps_u[:], mybir.ActivationFunctionType.Relu
                    )
                else:
                    nc.vector.tensor_scalar_max(t_h[:], ps_u[:], 0.0)
                h_tiles[n] = t_h

            def back(n):
                """mix matmuls + 672/352 copies (+ DMA per 4 groups)."""
                jt, g = divmod(n, NG)
                t_h = h_tiles.pop(n)
                if g % 4 == 0:
                    to_cur[0] = op.tile([128, 8 * J], f16, tag="t_o", name="t_o")
                t_o = to_cur[0]
                ps_o = psm.tile([128, 2 * J], f32, tag="ps_o")
                for w in range(2):
                    pr = slice(64 * w, 64 * w + 64)
                    nc.tensor.matmul(
                        ps_o[:, w * J:(w + 1) * J], t_mixw[pr, :], t_h[pr, :],
                        start=True, stop=True,
                    )
                c0 = (g % 4) * 2 * J
                nc.scalar.copy(t_o[:, c0:c0 + 672], ps_o[:, 0:672])
                nc.vector.tensor_copy(t_o[:, c0 + 672:c0 + 1024], ps_o[:, 672:1024])
                jcol = slice(jt * J, (jt + 1) * J)
                if n >= NSTEP - 4:
                    # tail: DMA per group so the drain is short
                    row0 = g * 8 * GR
                    dview = scratch[row0:row0 + 256, jcol].rearrange(
                        "(w p) j -> p w j", w=2
                    )
                    src = t_o[:, c0:c0 + 2 * J].rearrange(
                        "p (w j) -> p w j", w=2
                    )
                    nc.gpsimd.dma_start(dview, src)
                elif g % 4 == 3:
                    row0 = (g - 3) * 8 * GR
                    dview = scratch[row0:row0 + 1024, jcol].rearrange(
                        "(g w p) j -> p g w j", g=4, w=2
                    )
                    src = t_o[:].rearrange("p (g w j) -> p g w j", g=4, w=2)
                    nc.gpsimd.dma_start(dview, src)

            # prologue: j-tile 0's S stages + per-pair basis rows so the
            # first u-matmuls start after pair 0, not after all four
            rt_alloc(0)
            for p in range(4):
                ps = s_pair_mm(0, p)
                s_pair_sqrt(0, p, ps)
                basis_dmas(0, p)

            # steady state: fronts lead backs by 2 groups; jt+1's S pairs
            # and basis DMAs interleave at fixed steps.  The sqrt for a
            # pair is emitted after the step's copies so it never blocks
            # them in the ACT queue.
            sq_pend = [None]
            for n in range(NSTEP + 2):
                jt, g = divmod(n, NG) if n < NSTEP else (NJ - 1, 16)
                if n < NSTEP:
                    if jt + 1 < NJ:
                        if g == 1:
                            rt_alloc(jt + 1)
                        if g in (1, 5, 9, 12):
                            p = {1: 0, 5: 1, 9: 2, 12: 3}[g]
                            sq_pend[0] = (jt + 1, p, s_pair_mm(jt + 1, p))
                        if g == 13:
                            basis_dmas(jt + 1)
                    front(n)
                if n >= 2:
                    back(n - 2)
                if sq_pend[0] is not None:
                    s_pair_sqrt(*sq_pend[0])
                    sq_pend[0] = None

    nc.compile()
    return nc


def _host_inputs(node_coords, W1, b1, W2, b2):
    coords = node_coords.astype(np.float32)
    W1 = W1.astype(np.float32)
    b1 = b1.astype(np.float32)
    W2 = W2.astype(np.float32)

    a = W1[:, 3]                       # [3] dist coefficients
    Wc = W1[:, :3]                     # [3,3] coord coefficients
    g = coords @ Wc.T                  # [N,3]
    c2 = (coords * coords).sum(1)      # [N]

    # ---- S rhs: j-side basis rows [cx, cy, cz, |c|^2, 1], split2 ----
    R = np.zeros((5, N), np.float32)
    R[0:3] = coords.T
    R[3] = c2
    R[4] = 1.0
    Rh, Rl = _split2(R)
    s_rhs = np.vstack([Rh, Rl, Rh])                   # [15, N]

    # ---- basis rows for the u-matmul rhs: [1, g0, g1, g2], replicated
    # over the 8 S-tile column blocks; rows 4-7 duplicate for slot 96 ----
    basis = np.zeros((4, N), np.float32)
    basis[0] = 1.0
    basis[1:4] = g.T
    basisD = np.empty((8, NJ * NT * J), np.float32)
    bv = basisD.reshape(2, 4, NJ, NT, J)
    for t in range(NT):
        bv[:, :, :, t, :] = basis.reshape(4, NJ, J)
    basisD = basisD.astype(_BF)

    # ---- mix weights (block-diagonal W2), two 64-row windows ----
    mixw = np.zeros((128, 128), np.float32)
    for w in range(2):
        for di in range(16):
            for m in range(3):
                mixw[64 * w + 4 * di + m, 8 * di + 0:8 * di + 8] = W2[:, m]
    mixw = mixw.astype(np.float16)

    in_maps = []
    for c in range(N_CORES):
        r = coords[c * ROWS:(c + 1) * ROWS]          # [512,3]
        r2 = (r * r).sum(1)                          # [512]
        alpha = r @ Wc.T + b1                        # [512,3]

        # ---- S lhsT: tile t holds groups 2t,2t+1 at col offsets
        # 0/36 within its 128-col block; basis cols 32-35 stay zero ----
        L = np.zeros((5, NT * 128), np.float32)
        for t in range(NT):
            for s in range(2):
                gi = 2 * t + s
                i0 = gi * GR
                cc = t * 128 + _SLOT_OFF[s]
                rr = r[i0:i0 + GR]                   # [32,3]
                L[0:3, cc:cc + GR] = -2.0 * rr.T
                L[3, cc:cc + GR] = 1.0
                L[4, cc:cc + GR] = r2[i0:i0 + GR] + EPS
        Lh, Ll = _split2(L)
        s_lhsT = np.vstack([Lh, Lh, Ll])             # [15, NT*128]

        # ---- u lhsT [100, NG*128]: rows live at the group's slot
        # partitions so lhsT/rhs share a base partition.  Per slot:
        # dist rows off..off+31 (a), ones row off+32 (alpha), g rows
        # off+33..35 (-1) ----
        u = np.zeros((100, NG * 128), np.float32)
        for gi in range(NG):
            off = _SLOT_OFF[gi % 2]
            i0 = gi * GR
            for di in range(GR):
                for m in range(3):
                    p = gi * 128 + 4 * di + m
                    u[off + di, p] = a[m]
                    u[off + 32, p] = alpha[i0 + di, m]
                    u[off + 33 + m, p] = -1.0
        u_lhsT = u.astype(_BF)

        in_maps.append({
            "s_lhsT": np.ascontiguousarray(s_lhsT),
            "s_rhs": np.ascontiguousarray(s_rhs),
            "basisD": np.ascontiguousarray(basisD),
            "u_lhsT": np.ascontiguousarray(u_lhsT),
            "mixw": mixw,
        })
    return in_maps


def kernel(node_coords, W1, b1, W2, b2):
    from concourse.bass_utils import run_bass_kernel_spmd

    if "nc" not in _CACHE:
        _CACHE["nc"] = _build_program()
    nc = _CACHE["nc"]

    in_maps = _host_inputs(node_coords, W1, b1, W2, b2)
    res = run_bass_kernel_spmd(nc, in_maps, core_ids=list(range(N_CORES)))
    _CACHE["last_res"] = res

    out = np.empty((N, N, DIM_OUT), np.float32)
    for c in range(N_CORES):
        sc = res.results[c]["scratch"]                   # [4096, 4096] f16
        blk = sc.astype(np.float32).reshape(ROWS, DIM_OUT, N).transpose(0, 2, 1)
        out[c * ROWS:(c + 1) * ROWS] = blk

    b2f = b2.astype(np.float32)
    if np.any(b2f):
        out += b2f

    # exact diagonal (device path has an eps floor under the sqrt)
    h_diag = np.maximum(b1.astype(np.float32), 0.0)
    diag = W2.astype(np.float32) @ h_diag + b2f
    idx = np.arange(N)
    out[idx, idx, :] = diag

    return out
